# revision 22
# baseline (speedup 1.0000x reference)
"""DA-RNN (dual-stage attention RNN) Trainium2 kernel, 8-core SPMD,
data-parallel over batch (b=64/core). Self-contained: builds + compiles the
Bass kernel at call time and runs via run_bass_kernel_spmd.

v2 design notes (vs baseline):
  - LSTM gates fully accumulated in PSUM: x-part, bias (K=1 ones-matmul) and
    decoder y-outer-product all ride the PE; the activation reads PSUM
    directly. No DVE gate adds.
  - h/c states stored DOUBLED (H=2h, C=2c); every weight consuming a state is
    host-prescaled by 0.5. sigmoid(x)=(tanh(x/2)+1)/2 then folds into
    scalar_tensor_tensor ops: one tanh ACT per layer covers all gates
    (g-gate weights prescaled x2), and the cell/hidden updates are 4 fused
    stt ops.
  - gate chunk order (i,i,g,g,f,f,o,o) so the gate ACT can be split in
    halves pipelined against the second kc-half of the matmuls.
  - decoder: per-step context is never materialized. q = wc.Xe precomputed;
    z = sum_tau beta*q via ONE tensor_tensor_reduce with initial=yc. Full
    context computed once after the loop for the final FC.
  - attention tensors in tau-major layout [128, 2, T, b]; Hpre = PX + ph
    broadcast add in bf16; processed in 2 batch-chunks pipelined across
    DVE(add) / ACT(tanh) / PE(w2 contraction) / DMA / ACT(exp).
  - decoder BatchNorm stats: z -> PE transpose -> sums -> 8-byte AllGather;
    rstd via bit-trick rsqrt; LSTM h-matmuls pre-issued so the PE works
    through the collective window.
"""
import numpy as np
import concourse.bass as bass
import concourse.tile as tile
import concourse.mybir as mybir
from concourse.masks import make_identity
from concourse.bass_utils import run_bass_kernel_spmd

F32 = mybir.dt.float32
BF16 = mybir.dt.bfloat16
I32 = mybir.dt.int32
AF = mybir.ActivationFunctionType
ALU = mybir.AluOpType
AX = mybir.AxisListType


def split_multiwait(nc, max_waits=1):
    """walrus on this container only allows 1 sync-wait per instruction;
    hoist extras onto nofuse NoOps on the same engine queue."""
    for fn in nc.m.functions:
        for blk in fn.blocks:
            newlist = []
            for inst in blk.instructions:
                si = getattr(inst, 'sync_info', None)
                if si is not None and si.on_wait and len(si.on_wait) > max_waits:
                    waits = list(si.on_wait)
                    si.on_wait = waits[-max_waits:]
                    extra = waits[:-max_waits]
                    for j in range(0, len(extra), max_waits):
                        nop = mybir.InstNoOp(
                            name=f"{inst.name}-wsplit{j}", ins=[], outs=[],
                            sync_info=mybir.SyncInfo(
                                on_wait=extra[j:j + max_waits], on_update=[]),
                            bass_nofuse=True)
                        nop.engine = inst.engine
                        newlist.append(nop)
                newlist.append(inst)
            blk.instructions = newlist
    return nc


def emit_rsqrt(nc, pool, out_ap, y_ap, shape, iters=2):
    """out = 1/sqrt(y) elementwise via bit-trick seed + Newton (DVE only)."""
    p, f = shape
    t_int = pool.tile([p, f], I32, tag="rsqrt_int")
    r = pool.tile([p, f], F32, tag="rsqrt_r")
    s = pool.tile([p, f], F32, tag="rsqrt_s")
    nc.vector.tensor_scalar(out=t_int[:], in0=y_ap.bitcast(I32), scalar1=1,
                            scalar2=None, op0=ALU.logical_shift_right)
    nc.vector.tensor_scalar(out=t_int[:], in0=t_int[:], scalar1=-1,
                            scalar2=0x5F3759DF, op0=ALU.mult, op1=ALU.add)
    nc.vector.tensor_copy(out=r[:], in_=t_int[:].bitcast(F32))
    for _ in range(iters):
        nc.vector.tensor_mul(out=s[:], in0=r[:], in1=r[:])
        nc.vector.tensor_mul(out=s[:], in0=s[:], in1=y_ap)
        nc.vector.tensor_scalar(out=s[:], in0=s[:], scalar1=-0.5, scalar2=1.5,
                                op0=ALU.mult, op1=ALU.add)
        nc.vector.tensor_mul(out=r[:], in0=r[:], in1=s[:])
    nc.vector.tensor_copy(out=out_ap, in_=r[:])


def run8(nc, in_maps, trace=False):
    split_multiwait(nc)
    return run_bass_kernel_spmd(nc, in_maps, list(range(8)), trace=trace)


B, T, N, HE, HD = 512, 32, 128, 256, 256
b = 64   # per-core batch
bc = 32  # attention batch-chunk
NCH = b // bc
EPS = 1e-5
GROUPS = [list(range(8))]

# gate chunk order (i,i,g,g,f,f,o,o): torch rows (i 0:256, f 256:512,
# g 512:768, o 768:1024)
PERM = np.concatenate([np.arange(0, 256), np.arange(512, 768),
                       np.arange(256, 512), np.arange(768, 1024)])
# post-PERM row scaling: g-gates (rows 256:512 after PERM) doubled so one
# tanh(0.5*x) ACT covers sigmoid-form and tanh-form gates
GSCALE = np.ones(1024, np.float32)
GSCALE[256:512] = 2.0
# chain slices after PERM: i=chunks 0:2, g=2:4, f=4:6, o=6:8
SL_I, SL_G, SL_F, SL_O = (0, 2), (2, 4), (4, 6), (6, 8)


def bcast(ap, pos, count):
    """Insert a stride-0 axis of length `count` at free-dim position `pos`."""
    a = [list(x) for x in ap.ap]
    a.insert(1 + pos, [0, count])
    return bass.AP(tensor=ap.tensor, offset=ap.offset, ap=a)


def bmajor(ap):
    """[p, T, b] view (tau-major storage) -> [p, (b T)] b-major-cols AP."""
    part, (st_t, n_t), (st_b, n_b) = ap.ap
    return bass.AP(tensor=ap.tensor, offset=ap.offset,
                   ap=[list(part), [st_b, n_b], [st_t, n_t]])


def host_prep(inputs):
    import ml_dtypes
    bf = ml_dtypes.bfloat16
    f = np.float32
    d = {k: np.asarray(v) for k, v in inputs.items()}

    def fold(WT):
        # [K, M] -> [128, K//128, M]
        K, M = WT.shape
        if K < 128:
            return None  # decoder l0 input column handled via dW0i_pm
        return np.ascontiguousarray(WT.reshape(K // 128, 128, M).transpose(1, 0, 2))

    def lstm(Wi, Wh, bi, bh, half_i):
        # rows permuted to (i,i,g,g,f,f,o,o); g rows doubled; h-consumers halved
        Wi2 = d[Wi][PERM] * GSCALE[:, None] * (0.5 if half_i else 1.0)
        Wh2 = d[Wh][PERM] * GSCALE[:, None] * 0.5
        bias = (d[bi] + d[bh])[PERM] * GSCALE
        return (fold(Wi2.T.astype(bf)), fold(Wh2.T.astype(bf)),
                np.ascontiguousarray(bias.astype(bf)[None, :]))

    sh = {}
    sh['eW0iT'], sh['eW0hT'], sh['eb0'] = lstm('eW0i', 'eW0h', 'eb0i', 'eb0h', False)
    sh['eW0iT'] = sh['eW0iT'].reshape(128, 1, 1024)
    sh['eW1iT'], sh['eW1hT'], sh['eb1'] = lstm('eW1i', 'eW1h', 'eb1i', 'eb1h', True)
    _, sh['dW0hT'], sh['db0'] = lstm('dW0i', 'dW0h', 'db0i', 'db0h', False)
    sh['dW1iT'], sh['dW1hT'], sh['db1'] = lstm('dW1i', 'dW1h', 'db1i', 'db1h', True)
    # decoder l0 input column (input y_tilde, not a doubled state)
    sh['dW0i_pm'] = np.ascontiguousarray(
        (d['dW0i'][PERM, 0] * GSCALE).astype(bf)[None, :])

    W1 = d['dattn_W1'] * 0.5  # h1, c1, Xe all doubled
    sh['W1hT'] = fold(W1[:, :HD].T.astype(bf))
    sh['W1cT'] = fold(W1[:, HD:2 * HD].T.astype(bf))
    sh['W1xT'] = fold(W1[:, 2 * HD:].T.astype(bf))
    sh['b1'] = np.ascontiguousarray(d['dattn_b1'].astype(f).reshape(2, 128).T)
    sh['w2c'] = np.ascontiguousarray(d['dattn_W2'][0].astype(bf).reshape(2, 128).T)
    sh['wc'] = np.ascontiguousarray(
        (d['fc_W'][0, :HE] * 0.5).astype(bf).reshape(2, 128).T)
    sh['fcfh'] = np.ascontiguousarray(
        (d['fcf_W'][0, :HD] * 0.5).astype(bf).reshape(2, 128).T)
    sh['fcfc'] = np.ascontiguousarray(
        (d['fcf_W'][0, HD:] * 0.5).astype(bf).reshape(2, 128).T)
    # scalars: w_y, fc_b, gB, -g, fcbn_b, fcf_b
    g_ = float(d['fcbn_g'][0])
    sh['scal'] = np.array([[float(d['fc_W'][0, HE]), float(d['fc_b'][0]),
                            g_ * B, -g_, float(d['fcbn_b'][0]),
                            float(d['fcf_b'][0]), 0, 0]], f)
    sh['wXb'] = np.ascontiguousarray(
        np.broadcast_to(d['enc_attn_W'][0, 2 * HE:].astype(f), (128, T)))
    sh['bn1_g'] = np.ascontiguousarray(d['bn1_g'].astype(f)[:, None])
    sh['bn1_b'] = np.ascontiguousarray(d['bn1_b'].astype(f)[:, None])

    cores = []
    X = d['X'].astype(f)
    yp = d['y_prev'].astype(f)[:, :, 0]
    for c in range(8):
        sl = slice(c * b, (c + 1) * b)
        m = dict(sh)
        m['XTt'] = np.ascontiguousarray(X[sl].transpose(2, 0, 1))  # [128, b, T]
        m['XTb'] = np.ascontiguousarray(X[sl].transpose(2, 1, 0))  # [128, T, b]
        m['ypf'] = np.ascontiguousarray(
            yp[sl] * d['fc_W'][0, HE] + d['fc_b'][0]).astype(f)    # yc [b, T]
        cores.append(m)
    return cores


def declare_params(nc):
    P = {}
    def di(name, shape, dt=F32):
        P[name] = nc.declare_dram_parameter(name, list(shape), dt, isOutput=False)
    di('XTt', (128, b, T)); di('XTb', (128, T, b)); di('ypf', (b, T))
    di('eW0iT', (128, 1, 1024), BF16); di('eW0hT', (128, 2, 1024), BF16)
    di('eW1iT', (128, 2, 1024), BF16); di('eW1hT', (128, 2, 1024), BF16)
    di('dW0hT', (128, 2, 1024), BF16)
    di('dW1iT', (128, 2, 1024), BF16); di('dW1hT', (128, 2, 1024), BF16)
    di('eb0', (1, 1024), BF16); di('eb1', (1, 1024), BF16)
    di('db0', (1, 1024), BF16); di('db1', (1, 1024), BF16)
    di('dW0i_pm', (1, 1024), BF16)
    di('W1hT', (128, 2, 256), BF16); di('W1cT', (128, 2, 256), BF16)
    di('W1xT', (128, 2, 256), BF16)
    di('b1', (128, 2)); di('w2c', (128, 2), BF16); di('wc', (128, 2), BF16)
    di('fcfh', (128, 2), BF16); di('fcfc', (128, 2), BF16); di('scal', (1, 8))
    di('wXb', (128, T)); di('bn1_g', (128, 1)); di('bn1_b', (128, 1))
    return P


def lstm_chain(nc, pool, Sh, C, Hout, tag=""):
    """Fused LSTM pointwise update with doubled states.
    Sh = tanh(0.5*gates) over chunks (i,i,g,g,f,f,o,o) [128, 8, b].
    C (f32, doubled) updated in place; Hout (bf16, doubled) written."""
    t2 = pool.tile([128, 2, b], F32, tag="lt2")
    t1 = pool.tile([128, 2, b], F32, tag="lt1")
    nc.vector.scalar_tensor_tensor(out=t2[:], in0=Sh[:, SL_I[0]:SL_I[1], :],
                                   scalar=1.0, in1=Sh[:, SL_G[0]:SL_G[1], :],
                                   op0=ALU.add, op1=ALU.mult)
    nc.vector.scalar_tensor_tensor(out=t1[:], in0=Sh[:, SL_F[0]:SL_F[1], :],
                                   scalar=1.0, in1=C[:], op0=ALU.add, op1=ALU.mult)
    nc.vector.scalar_tensor_tensor(out=C[:], in0=t1[:], scalar=0.5, in1=t2[:],
                                   op0=ALU.mult, op1=ALU.add)
    Tc = pool.tile([128, 2, b], F32, tag="lTc")
    nc.scalar.activation(out=Tc[:], in_=C[:], func=AF.Tanh, scale=0.5)
    nc.vector.scalar_tensor_tensor(out=Hout, in0=Sh[:, SL_O[0]:SL_O[1], :],
                                   scalar=1.0, in1=Tc[:], op0=ALU.add, op1=ALU.mult)


def build(stage="full"):
    nc = bass.Bass(num_devices=8)
    P = declare_params(nc)
    out_y = nc.declare_dram_parameter("y_out", [1, b], F32, isOutput=True)
    dbg = {}
    if stage == "enc":
        dbg['Xenc'] = nc.declare_dram_parameter("dbg_xenc", [128, 2, T, b], F32,
                                                isOutput=True)
        dbg['H0h'] = nc.declare_dram_parameter("dbg_h0h", [128, 2, T, b], F32,
                                               isOutput=True)
        dbg['G1'] = nc.declare_dram_parameter("dbg_g1", [128, 8, b], F32,
                                              isOutput=True)

    with tile.TileContext(nc) as tc:
        import contextlib
        with contextlib.ExitStack() as ctx:
            singles = ctx.enter_context(tc.tile_pool(name="singles", bufs=1))
            pool = ctx.enter_context(tc.tile_pool(name="small", bufs=2))
            dpool = ctx.enter_context(tc.tile_pool(name="dram", bufs=1, space="DRAM"))

            S = {}
            for name, t in P.items():
                if name in ('XTt', 'XTb'):
                    continue
                shp = [int(x) for x in t.shape]
                S[name] = singles.tile(shp, t.dtype, name=name, tag=name)
                nc.sync.dma_start(out=S[name][:], in_=t[:])

            ones_bf = singles.tile([1, 128], BF16)
            nc.vector.memset(ones_bf[:], 1.0)
            ident = singles.tile([128, 128], F32)
            make_identity(nc, ident[:])

            h0T = singles.tile([128, 2, b], BF16)
            h1T = singles.tile([128, 2, b], BF16)
            c0 = singles.tile([128, 2, b], F32)
            c1 = singles.tile([128, 2, b], F32)
            Xe = singles.tile([128, 2, T, b], BF16)   # encoder h1 (doubled), tau-major
            H0h = singles.tile([128, 2, T, b], F32, name="H0h") if stage == "enc" else None
            G1d = singles.tile([128, 8, b], F32, name="G1d") if stage == "enc" else None
            xb = singles.tile([128, T, b], BF16)      # normalized attn-weighted input
            for st in (h0T, c0, c1):
                nc.vector.memset(st[:], 0.0)

            # ================= encoder prolog =================
            with tc.tile_pool(name="psA", bufs=2, space="PSUM") as psA, \
                 tc.tile_pool(name="pbig", bufs=1) as pbig:
                XTt = pbig.tile([128, b, T], F32, tag="bigA")
                nc.sync.dma_start(out=XTt[:], in_=P['XTt'][:])
                prod = pbig.tile([128, b, T], F32, tag="bigB")
                nc.vector.tensor_mul(out=prod[:], in0=XTt[:], in1=bcast(S['wXb'][:], 0, b))
                epreT = pool.tile([128, b], F32, tag="epreT")
                nc.vector.tensor_reduce(out=epreT[:], in_=prod[:], axis=AX.X, op=ALU.add)
                ps_e = psA.tile([b, 128], F32, tag="pse")
                nc.tensor.transpose(ps_e[:], epreT[:], ident[:])
                mx = pool.tile([b, 1], F32, tag="mx")
                nc.vector.tensor_reduce(out=mx[:], in_=ps_e[:], axis=AX.X, op=ALU.max,
                                        negate=True)
                ex = pool.tile([b, 128], F32, tag="ex")
                sm = pool.tile([b, 1], F32, tag="sm")
                nc.scalar.activation(out=ex[:], in_=ps_e[:], func=AF.Exp, bias=mx[:],
                                     scale=1.0, accum_out=sm[:])
                rs = pool.tile([b, 1], F32, tag="rs")
                nc.vector.reciprocal(out=rs[:], in_=sm[:])
                alpha = pool.tile([b, 128], F32, tag="alpha")
                nc.vector.tensor_scalar_mul(alpha[:], ex[:], rs[:])
                ps_a = psA.tile([128, b], F32, tag="psa")
                nc.tensor.transpose(ps_a[:], alpha[:], ident[0:b, 0:b])
                alphaT = pool.tile([128, b], F32, tag="alphaT")
                nc.vector.tensor_copy(out=alphaT[:], in_=ps_a[:])

                XTb = pbig.tile([128, T, b], F32, tag="bigA")
                nc.sync.dma_start(out=XTb[:], in_=P['XTb'][:])
                xt = pbig.tile([128, T, b], F32, tag="bigB")
                nc.vector.tensor_mul(out=xt[:], in0=XTb[:], in1=bcast(alphaT[:], 0, T))
                stats = pool.tile([128, 2, T], F32, tag="stats")
                nc.vector.tensor_reduce(out=stats[:, 0, :], in_=xt[:], axis=AX.X, op=ALU.add)
                sq = pbig.tile([128, T, b], F32, tag="bigA")
                nc.scalar.activation(out=sq[:], in_=xt[:], func=AF.Square)
                nc.vector.tensor_reduce(out=stats[:, 1, :], in_=sq[:], axis=AX.X, op=ALU.add)

                arin = dpool.tile([128, 2 * T], F32, tag="arin")
                arout = nc.dram_tensor("arout", [128, 2 * T], F32, addr_space="Shared")
                nc.sync.dma_start(out=arin[:], in_=stats[:])
                nc.gpsimd.collective_compute("AllReduce", ALU.add, replica_groups=GROUPS,
                                             ins=[arin[:]], outs=[arout[:]])
                stot = pool.tile([128, 2, T], F32, tag="stot")
                nc.sync.dma_start(out=stot[:], in_=arout[:])
                m = pool.tile([128, T], F32, tag="m")
                nc.vector.tensor_scalar_mul(m[:], stot[:, 0, :], 1.0 / B)
                v = pool.tile([128, T], F32, tag="v")
                msq = pool.tile([128, T], F32, tag="msq")
                nc.vector.tensor_mul(out=msq[:], in0=m[:], in1=m[:])
                nc.vector.tensor_scalar(out=v[:], in0=stot[:, 1, :], scalar1=1.0 / B,
                                        scalar2=EPS, op0=ALU.mult, op1=ALU.add)
                nc.vector.tensor_sub(out=v[:], in0=v[:], in1=msq[:])
                rstd = pool.tile([128, T], F32, tag="rstd")
                emit_rsqrt(nc, pool, rstd[:], v[:], [128, T], iters=2)
                A = pool.tile([128, T], F32, tag="A")
                nc.vector.tensor_scalar(out=A[:], in0=rstd[:], scalar1=S['bn1_g'][:, 0:1],
                                        scalar2=None, op0=ALU.mult)
                mA = pool.tile([128, T], F32, tag="mA")
                nc.vector.tensor_mul(out=mA[:], in0=m[:], in1=A[:])
                u = pbig.tile([128, T, b], F32, tag="bigA")
                nc.vector.tensor_mul(out=u[:], in0=xt[:], in1=bcast(A[:], 1, b))
                nc.vector.tensor_sub(out=u[:], in0=u[:], in1=bcast(mA[:], 1, b))
                nc.vector.tensor_scalar(out=xb[:], in0=u[:], scalar1=S['bn1_b'][:, 0:1],
                                        scalar2=None, op0=ALU.add)

            # ================= encoder loop =================
            # PSUM accumulation discipline: exactly ONE start (first matmul
            # into the tile) and ONE stop (last) per tile generation --
            # start=True resets has-written bits bank-wide, so interleaved
            # per-region groups lose earlier contributions.
            class Phase:
                def __init__(self, total):
                    self.total, self.i = total, 0
                def mm(self, out, l, r):
                    nc.tensor.matmul(out, l, r, start=(self.i == 0),
                                     stop=(self.i == self.total - 1))
                    self.i += 1

            with tc.tile_pool(name="psB", bufs=2, space="PSUM") as psB:
                for t in range(T):
                    # ---- layer 0: closed per-gc groups (LDW pipelining) ----
                    g0 = psB.tile([128, 8, b], F32, tag="g")
                    for gc in range(8):
                        gs = slice(gc * 128, (gc + 1) * 128)
                        grp = [(S['eb0'][0:1, gs], ones_bf[0:1, 0:b]),
                               (S['eW0iT'][:, 0, gs], xb[:, t, :])]
                        if t > 0:
                            for kc in range(2):
                                grp.append((S['eW0hT'][:, kc, gs], h0T[:, kc, :]))
                        for i, (l, r) in enumerate(grp):
                            nc.tensor.matmul(g0[:, gc, :], l, r, start=(i == 0),
                                             stop=(i == len(grp) - 1))
                    Sh0 = pool.tile([128, 8, b], F32, tag="Sh")
                    nc.scalar.activation(out=Sh0[:], in_=g0[:], func=AF.Tanh, scale=0.5)
                    # ---- layer 0 pointwise ----
                    lstm_chain(nc, pool, Sh0, c0, h0T[:])
                    if stage == "enc":
                        nc.vector.tensor_copy(out=H0h[:, :, t, :], in_=h0T[:])
                    # ---- layer 1: closed per-gc groups after h0 ----
                    g1 = psB.tile([128, 8, b], F32, tag="g")
                    for gc in range(8):
                        gs = slice(gc * 128, (gc + 1) * 128)
                        grp = [(S['eb1'][0:1, gs], ones_bf[0:1, 0:b])]
                        if t > 0:
                            for kc in range(2):
                                grp.append((S['eW1hT'][:, kc, gs], Xe[:, kc, t - 1, :]))
                        for kc in range(2):
                            grp.append((S['eW1iT'][:, kc, gs], h0T[:, kc, :]))
                        for i, (l, r) in enumerate(grp):
                            nc.tensor.matmul(g1[:, gc, :], l, r, start=(i == 0),
                                             stop=(i == len(grp) - 1))
                    Sh1 = pool.tile([128, 8, b], F32, tag="Sh")
                    nc.scalar.activation(out=Sh1[:], in_=g1[:], func=AF.Tanh, scale=0.5)
                    if stage == "enc" and t == 1:
                        nc.vector.tensor_copy(out=G1d[:], in_=g1[:])
                    lstm_chain(nc, pool, Sh1, c1, Xe[:, :, t, :])

            if stage == "enc":
                with tc.tile_pool(name="dbgp", bufs=1) as dbgp:
                    xef = dbgp.tile([128, 2, T, b], F32)
                    nc.vector.tensor_copy(out=xef[:], in_=Xe[:])
                    nc.sync.dma_start(out=dbg['Xenc'][:], in_=xef[:])
                    nc.sync.dma_start(out=dbg['H0h'][:], in_=H0h[:])
                    nc.sync.dma_start(out=dbg['G1'][:], in_=G1d[:])
                    yz = pool.tile([1, b], F32, tag="yz")
                    nc.vector.memset(yz[:], 0.0)
                    nc.sync.dma_start(out=out_y[:], in_=yz[:])
                return nc

            # ================= decoder prolog =================
            PX = singles.tile([128, 2, T, b], BF16)
            q = singles.tile([b, T], F32)
            yc = singles.tile([b, T], F32)
            nc.sync.dma_start(out=yc[:], in_=P['ypf'][:])
            with tc.tile_pool(name="psPX", bufs=1, space="PSUM") as psPX:
                for g2 in range(2):
                    pxp = psPX.tile([128, T * b], F32, tag="pxp")
                    for qq in range(4):
                        for kc in range(2):
                            src = Xe[:, kc, :, :].rearrange("p t b2 -> p (t b2)")
                            nc.tensor.matmul(
                                pxp[:, qq * 512:(qq + 1) * 512],
                                S['W1xT'][:, kc, g2 * 128:(g2 + 1) * 128],
                                src[:, qq * 512:(qq + 1) * 512],
                                start=(kc == 0), stop=(kc == 1))
                    nc.vector.tensor_scalar(
                        out=PX[:, g2, :, :].rearrange("p t b2 -> p (t b2)"), in0=pxp[:],
                        scalar1=S['b1'][:, g2:g2 + 1], scalar2=None, op0=ALU.add)
                # q = wc . Xe  -> [1, b*T] (b-major cols) -> DMA to [b, T]
                pq = psPX.tile([128, T * b], F32, tag="pxp")
                for qq in range(4):
                    for kc in range(2):
                        src = bmajor(Xe[:, kc, :, :])
                        nc.tensor.matmul(pq[0:1, qq * 512:(qq + 1) * 512],
                                         S['wc'][:, kc:kc + 1],
                                         src[:, qq * 16:(qq + 1) * 16, :],
                                         start=(kc == 0), stop=(kc == 1))
                qsb = pool.tile([1, T * b], F32, tag="qsb", bufs=1)
                nc.vector.tensor_copy(out=qsb[:], in_=pq[0:1, :])
                nc.sync.dma_start(out=q[:], in_=qsb[:])

            for st in (h0T, h1T, c0, c1):
                nc.vector.memset(st[:], 0.0)
            c1bf = singles.tile([128, 2, b], BF16)
            nc.vector.memset(c1bf[:], 0.0)
            betaf = singles.tile([b, T], BF16)
            z = singles.tile([b, 1], F32)
            h1f = singles.tile([128, 2, b], BF16)

            # ================= decoder loop =================
            with tc.tile_pool(name="psG", bufs=2, space="PSUM") as psG, \
                 tc.tile_pool(name="psPh", bufs=1, space="PSUM") as psPh, \
                 tc.tile_pool(name="psE", bufs=1, space="PSUM") as psE, \
                 tc.tile_pool(name="psM", bufs=1, space="PSUM") as psM, \
                 tc.tile_pool(name="att", bufs=2) as att:
                for t in range(T):
                    # ---- ph = W1h h1 + W1c c1 (t>0) ----
                    if t > 0:
                        ph = psPh.tile([128, 2, b], F32, tag="ph")
                        idx = 0
                        for g2 in range(2):
                            for W, R in ((S['W1cT'], c1bf), (S['W1hT'], h1T)):
                                for kc in range(2):
                                    nc.tensor.matmul(
                                        ph[:, g2, :],
                                        W[:, kc, g2 * 128:(g2 + 1) * 128],
                                        R[:, kc, :], start=(idx == 0), stop=(idx == 15))
                                    idx += 1
                        phS = pool.tile([128, 2, b], BF16, tag="phS")
                        nc.vector.tensor_copy(out=phS[:], in_=ph[:])
                    # ---- LSTM head matmuls (pre-run under attention+collective) ----
                    g0d = psG.tile([128, 8, b], F32, tag="g")
                    p0 = Phase(8 * (1 + (2 if t > 0 else 0)) + 8)
                    for gc in range(8):
                        gs = slice(gc * 128, (gc + 1) * 128)
                        p0.mm(g0d[:, gc, :], S['db0'][0:1, gs], ones_bf[0:1, 0:b])
                        if t > 0:
                            for kc in range(2):
                                p0.mm(g0d[:, gc, :], S['dW0hT'][:, kc, gs],
                                      h0T[:, kc, :])
                    g1d = psG.tile([128, 8, b], F32, tag="g")
                    p1 = Phase(8 * (1 + (2 if t > 0 else 0) + 2))
                    for gc in range(8):
                        gs = slice(gc * 128, (gc + 1) * 128)
                        p1.mm(g1d[:, gc, :], S['db1'][0:1, gs], ones_bf[0:1, 0:b])
                        if t > 0:
                            for kc in range(2):
                                p1.mm(g1d[:, gc, :], S['dW1hT'][:, kc, gs],
                                      h1T[:, kc, :])
                    # ---- attention, 2 batch-chunks pipelined ----
                    pe = psE.tile([1, T * b], F32, tag="pe")
                    esb = pool.tile([1, T * b], F32, tag="esb")
                    ebt = att.tile([b, T], F32, tag="ebt")
                    exd = att.tile([b, T], F32, tag="exd")
                    smd = att.tile([b, 1], F32, tag="smd")
                    rsd = att.tile([b, 1], F32, tag="rsd")
                    junk = att.tile([b, T], F32, tag="junk")
                    zr = att.tile([b, 1], F32, tag="zr")
                    for cch in range(NCH):
                        bsl = slice(cch * bc, (cch + 1) * bc)
                        if t > 0:
                            Hpre = att.tile([128, 2, T, bc], BF16, tag="Hpre")
                            nc.vector.tensor_add(
                                out=Hpre[:], in0=PX[:, :, :, bsl],
                                in1=bcast(phS[:, :, bsl], 1, T))
                            Hin = Hpre[:]
                        else:
                            Hin = PX[:, :, :, bsl]
                        Hc = att.tile([128, 2, T, bc], BF16, tag="Hc")
                        nc.scalar.activation(out=Hc[:], in_=Hin, func=AF.Tanh)
                        for hh in range(2):
                            for kc in range(2):
                                # contiguous tau-major cols (full-rate bf16 stream)
                                hsrc = Hc[:, kc, :, :].rearrange("p t b2 -> p (t b2)")
                                nc.tensor.matmul(
                                    pe[0:1, cch * 1024 + hh * 512:
                                       cch * 1024 + (hh + 1) * 512],
                                    S['w2c'][:, kc:kc + 1],
                                    hsrc[:, hh * 512:(hh + 1) * 512],
                                    start=(kc == 0), stop=(kc == 1))
                        nc.vector.tensor_copy(out=esb[:, cch * 1024:(cch + 1) * 1024],
                                              in_=pe[0:1, cch * 1024:(cch + 1) * 1024])
                        # tau-major -> DRAM -> transposed read -> [bc, T]
                        ed = dpool.tile([T, bc], F32, tag=f"ed{cch}")
                        nc.sync.dma_start(out=ed[:],
                                          in_=esb[:, cch * 1024:(cch + 1) * 1024])
                        edv = bass.AP(tensor=ed[:].tensor, offset=ed[:].offset,
                                      ap=[[1, bc], [bc, T]])
                        nc.sync.dma_start(out=ebt[bsl, :], in_=edv)
                        nc.scalar.activation(out=exd[bsl, :], in_=ebt[bsl, :],
                                             func=AF.Exp)
                        nc.vector.tensor_reduce(out=smd[bsl, :], in_=exd[bsl, :],
                                                axis=AX.X, op=ALU.add)
                        nc.vector.reciprocal(out=rsd[bsl, :], in_=smd[bsl, :])
                        nc.vector.tensor_scalar_mul(betaf[bsl, :], exd[bsl, :],
                                                    rsd[bsl, :])
                        nc.vector.tensor_mul(out=junk[bsl, :], in0=betaf[bsl, :],
                                             in1=q[bsl, :])
                        nc.vector.tensor_reduce(out=zr[bsl, :], in_=junk[bsl, :],
                                                axis=AX.X, op=ALU.add)
                        nc.vector.tensor_add(out=z[bsl, 0:1], in0=zr[bsl, :],
                                             in1=yc[bsl, t:t + 1])
                    # ---- z stats + collective ----
                    zT = psM.tile([1, b], F32, tag="zT")
                    nc.tensor.transpose(zT[:], z[:], ident[0:b, 0:b])
                    sqs = pool.tile([1, b], F32, tag="sqs")
                    nc.scalar.activation(out=sqs[:], in_=zT[:], func=AF.Square)
                    zs = pool.tile([1, 2], F32, tag="zs")
                    nc.vector.tensor_reduce(out=zs[:, 0:1], in_=zT[:], axis=AX.X,
                                            op=ALU.add)
                    nc.vector.tensor_reduce(out=zs[:, 1:2], in_=sqs[:], axis=AX.X,
                                            op=ALU.add)
                    agi = dpool.tile([1, 2], F32, tag="agi")
                    ago = nc.dram_tensor(f"ago{t}", [8, 2], F32, addr_space="Shared")
                    nc.sync.dma_start(out=agi[:], in_=zs[:])
                    nc.gpsimd.collective_compute("AllGather", ALU.bypass,
                                                 replica_groups=GROUPS,
                                                 ins=[agi[:]], outs=[ago[:]])
                    agf = pool.tile([1, 16], F32, tag="agf")
                    nc.sync.dma_start(out=agf[:], in_=ago[:])
                    Ssum = pool.tile([1, 2], F32, tag="Ssum")
                    agv = bass.AP(tensor=agf[:].tensor, offset=agf[:].offset,
                                  ap=[list(agf[:].ap[0]), [1, 2], [2, 8]])
                    nc.vector.tensor_reduce(out=Ssum[:], in_=agv, axis=AX.X, op=ALU.add)
                    S2B = pool.tile([1, 1], F32, tag="S2B")
                    nc.vector.tensor_scalar(out=S2B[:], in0=Ssum[:, 1:2],
                                            scalar1=float(B), scalar2=float(B) * B * EPS,
                                            op0=ALU.mult, op1=ALU.add)
                    S1sq = pool.tile([1, 1], F32, tag="S1sq")
                    nc.vector.tensor_scalar(out=S1sq[:], in0=Ssum[:, 0:1],
                                            scalar1=Ssum[:, 0:1], scalar2=None,
                                            op0=ALU.mult)
                    vb2 = pool.tile([1, 1], F32, tag="vb2")
                    nc.vector.tensor_sub(out=vb2[:], in0=S2B[:], in1=S1sq[:])
                    rv = pool.tile([1, 1], F32, tag="rv")
                    emit_rsqrt(nc, pool, rv[:], vb2[:], [1, 1], iters=1)
                    kk = pool.tile([1, 1], F32, tag="kk")
                    nc.vector.tensor_scalar(out=kk[:], in0=rv[:],
                                            scalar1=S['scal'][0:1, 2:3], scalar2=None,
                                            op0=ALU.mult)
                    rs1 = pool.tile([1, 1], F32, tag="rs1")
                    nc.vector.tensor_scalar(out=rs1[:], in0=rv[:], scalar1=Ssum[:, 0:1],
                                            scalar2=None, op0=ALU.mult)
                    ccb = pool.tile([1, 1], F32, tag="ccb")
                    nc.vector.tensor_scalar(out=ccb[:], in0=rs1[:],
                                            scalar1=S['scal'][0:1, 3:4],
                                            scalar2=S['scal'][0:1, 4:5],
                                            op0=ALU.mult, op1=ALU.add)
                    ytb = pool.tile([1, b], BF16, tag="ytb")
                    nc.vector.tensor_scalar(out=ytb[:], in0=zT[:], scalar1=kk[:, 0:1],
                                            scalar2=ccb[:, 0:1], op0=ALU.mult, op1=ALU.add)
                    # ---- l0 finish: y outer product (continues p0 phase) ----
                    for gc in range(8):
                        p0.mm(g0d[:, gc, :],
                              S['dW0i_pm'][0:1, gc * 128:(gc + 1) * 128],
                              ytb[0:1, :])
                    Sh0 = pool.tile([128, 8, b], F32, tag="Sh")
                    nc.scalar.activation(out=Sh0[:], in_=g0d[:], func=AF.Tanh, scale=0.5)
                    lstm_chain(nc, pool, Sh0, c0, h0T[:])
                    # ---- l1 finish: h0 matmuls in halves ----
                    Sh1 = pool.tile([128, 8, b], F32, tag="Sh")
                    for half in range(2):
                        for gc in range(half * 4, half * 4 + 4):
                            gs = slice(gc * 128, (gc + 1) * 128)
                            for kc in range(2):
                                p1.mm(g1d[:, gc, :], S['dW1iT'][:, kc, gs],
                                      h0T[:, kc, :])
                        nc.scalar.activation(out=Sh1[:, half * 4:half * 4 + 4, :],
                                             in_=g1d[:, half * 4:half * 4 + 4, :],
                                             func=AF.Tanh, scale=0.5)
                    if t == T - 1:
                        lstm_chain(nc, pool, Sh1, c1, h1f[:])
                    else:
                        lstm_chain(nc, pool, Sh1, c1, h1T[:])
                        nc.vector.tensor_copy(out=c1bf[:], in_=c1[:])

            # ================= final: context + fc =================
            with tc.tile_pool(name="psF", bufs=1, space="PSUM") as psF, \
                 tc.tile_pool(name="fin", bufs=1) as fin:
                # betaf [b, T] -> DRAM -> transposed read -> bflat[tau*b + b2]
                bflat = fin.tile([1, T * b], BF16)
                bd = dpool.tile([b, T], BF16, tag="bd")
                nc.sync.dma_start(out=bd[:], in_=betaf[:])
                bdv = bass.AP(tensor=bd[:].tensor, offset=bd[:].offset,
                              ap=[[1, T], [T, b]])
                nc.sync.dma_start(out=bflat[:], in_=bdv)
                pb = psF.tile([128, T * b], F32, tag="pb")
                for hh in range(4):
                    nc.tensor.matmul(pb[:, hh * 512:(hh + 1) * 512], ones_bf[0:1, :],
                                     bflat[0:1, hh * 512:(hh + 1) * 512],
                                     start=True, stop=True)
                ctxT = fin.tile([128, 2, b], F32)
                prodT = fin.tile([128, b, T], F32)
                for kc in range(2):
                    src = Xe[:, kc, :, :].rearrange("p t b2 -> p (t b2)")
                    pdst = bass.AP(tensor=prodT[:].tensor, offset=prodT[:].offset,
                                   ap=[list(prodT[:].ap[0]), [1, T], [T, b]])
                    nc.vector.tensor_mul(out=pdst, in0=src, in1=pb[:, :])
                    nc.vector.tensor_reduce(out=ctxT[:, kc, :], in_=prodT[:],
                                            axis=AX.X, op=ALU.add)
                ctxbf = fin.tile([128, 2, b], BF16)
                nc.vector.tensor_copy(out=ctxbf[:], in_=ctxT[:])
                pf = psF.tile([1, b], F32, tag="pf")
                idx = 0
                for W, R in ((S['fcfh'], h1f), (S['fcfc'], ctxbf)):
                    for kc in range(2):
                        nc.tensor.matmul(pf[:], W[:, kc:kc + 1], R[:, kc, :],
                                         start=(idx == 0), stop=(idx == 3))
                        idx += 1
                yv = pool.tile([1, b], F32, tag="yv")
                nc.scalar.activation(out=yv[:], in_=pf[:], func=AF.Relu,
                                     bias=S['scal'][0:1, 5:6], scale=1.0)
                nc.sync.dma_start(out=out_y[:], in_=yv[:])
    return nc


def kernel(**inputs) -> np.ndarray:
    cores = host_prep(inputs)
    nc = build(stage="full")
    res = run8(nc, cores)
    y = np.concatenate([res.results[c]["y_out"][0] for c in range(8)])[:, None]
    return y.astype(np.float32)
